# revision 1
# baseline (speedup 1.0000x reference)
"""Cross-attention Trainium2 kernel (Bass/Tile), 8-core SPMD.

Problem: B=2, Tq=Tk=2048, C=1024, H=16 heads, D=64.
  q = query @ Wq + bq ; k,v = context @ Wkv + bkv (split)
  out = softmax(q k^T / sqrt(D)) v  @ Wo + bo

Sharding (per the data-parallel-B x tensor-parallel-heads hint):
  core c in 0..7 handles batch b = c//4 and head group hg = c%4
  (4 consecutive heads = 256 channels). Each core computes the partial
  out-projection  O_local @ Wo[rows of its heads]  (+ bo/4) and the host
  sums the 4 partials per batch (row-parallel Wo reduction).

Device layout choices (per core):
  - query/context are supplied TRANSPOSED ([C, T]) so the C-contraction
    projections stream at full DMA rate with C on partitions.
  - q,k are produced transposed per head-pair: qt/kt [128=(2 heads x 64), T]
  - v is produced natural [Tk, (4 heads x 65)] with a ones-column per head
    so the P @ V matmul also accumulates the softmax denominator for free.
  - scores are computed per (head, 512-col q-block) in [Tk=128, Tq=512]
    chunks; exp on ScalarE (no max subtraction -- scores are O(5) here,
    exactly representable range for fp32 exp); P @ V accumulates
    OT' [65, 512] in PSUM where row 64 is the denominator.
  - normalization: reciprocal of row 64 (DVE), broadcast to 64 partitions
    with a K=1 ones matmul on the PE, multiply on DVE into ot [128, 512].
  - out-projection consumes ot directly as lhsT (c-dim on partitions) and
    produces the output in natural [Tq, C] layout for contiguous stores.
  All matmuls run as float32r (full PE rate at N>=256, fp32 storage).
"""

import numpy as np

import concourse.bass as bass
import concourse.mybir as mybir
import concourse.tile as tile
from concourse import bacc
from concourse.bass_utils import run_bass_kernel_spmd

F32 = mybir.dt.float32
F32R = mybir.dt.float32r
AF = mybir.ActivationFunctionType

T = 2048      # Tq = Tk
C = 1024      # embed dim
D = 64        # head dim
HL = 4        # heads per core
KT = C // 128  # 8 contraction tiles
NB = T // 512  # 4 blocks of 512
SCALE = float(D) ** -0.5

_PROGRAM = None


def _emit(tc):
    nc = tc.nc
    qT = nc.dram_tensor("qT", [C, T], F32R, kind="ExternalInput").ap()
    cT = nc.dram_tensor("cT", [C, T], F32R, kind="ExternalInput").ap()
    wq = nc.dram_tensor("wq", [C, 256], F32R, kind="ExternalInput").ap()
    wk = nc.dram_tensor("wk", [C, 256], F32R, kind="ExternalInput").ap()
    wv = nc.dram_tensor("wv", [C, 256], F32R, kind="ExternalInput").ap()
    wo = nc.dram_tensor("wo", [256, C], F32R, kind="ExternalInput").ap()
    bq = nc.dram_tensor("bq", [256], F32, kind="ExternalInput").ap()
    bk = nc.dram_tensor("bk", [256], F32, kind="ExternalInput").ap()
    bv = nc.dram_tensor("bv", [256], F32, kind="ExternalInput").ap()
    bo4 = nc.dram_tensor("bo4", [C], F32, kind="ExternalInput").ap()
    ones_in = nc.dram_tensor("ones_in", [64], F32R, kind="ExternalInput").ap()
    out = nc.dram_tensor("out", [T, C], F32, kind="ExternalOutput").ap()

    from contextlib import ExitStack

    with ExitStack() as ctx:
        consts = ctx.enter_context(tc.tile_pool(name="consts", bufs=1))
        acts = ctx.enter_context(tc.tile_pool(name="acts", bufs=1))

        # DMA dispatch costs ~650ns serial time on the dispatching engine's
        # sequencer, so: few large DMAs (one dma_start already stripes across
        # all 16 SDMA engines), loads on SP, stores + small biases on the
        # near-idle GpSimd sequencer.
        wk_sb = consts.tile([128, KT, 256], F32R, tag="wk")
        wk_r = wk.rearrange("(t p) m -> p t m", p=128)
        nc.sync.dma_start(out=wk_sb[:, 0:2, :], in_=wk_r[:, 0:2, :])
        nc.sync.dma_start(out=wk_sb[:, 2:KT, :], in_=wk_r[:, 2:KT, :])
        bk_sb = consts.tile([128, 2], F32, tag="bk")
        nc.gpsimd.dma_start(out=bk_sb, in_=bk.rearrange("(x p) -> p x", p=128))

        def _pbcast(ap):
            return bass.AP(
                tensor=ap.tensor, offset=ap.offset, ap=[[0, 128]] + list(ap.ap)
            )

        bv_bc = consts.tile([128, 256], F32, tag="bv")
        nc.gpsimd.dma_start(out=bv_bc, in_=_pbcast(bv))
        # ISA codegen rejects memset on float32r -- memset through an f32 view.
        ones64 = consts.tile([1, 64], F32R, tag="ones")
        nc.vector.memset(ones64.bitcast(F32), 1.0)
        # Warm the Exp activation table off the critical path (the first use
        # otherwise pays the ~2.7us table load at the start of attention).
        warm = consts.tile([1, 1], F32, tag="warm")
        nc.scalar.activation(warm, ones64.bitcast(F32)[:, 0:1], AF.Exp)

        # persistent projected activations
        qtb = [[acts.tile([128, 1024], F32R, tag=f"qt{J}{p}", name=f"qt{J}{p}")
                for p in range(2)] for J in range(NB // 2)]
        kt = [acts.tile([128, T], F32R, tag=f"kt{p}", name=f"kt{p}") for p in range(2)]
        vt = [acts.tile([128, HL, D + 1], F32R, tag=f"v{i}", name=f"v{i}") for i in range(T // 128)]

        # All SBUF pools live for the whole program: recycling SBUF across
        # the phase boundary makes the first phase-2 instruction on each
        # engine inherit WAW waits on all 8 DMA queues, which overflows the
        # ISA sync-wait table (walrus "Too many sync wait commands").
        ins_pool = ctx.enter_context(tc.tile_pool(name="ins", bufs=2))
        att = ctx.enter_context(tc.tile_pool(name="att", bufs=3))
        smo = ctx.enter_context(tc.tile_pool(name="smo", bufs=2))
        sm1 = ctx.enter_context(tc.tile_pool(name="sm1", bufs=1))
        otp = ctx.enter_context(tc.tile_pool(name="otp", bufs=1))
        outs_pool = ctx.enter_context(tc.tile_pool(name="outs", bufs=3))

        qT_r = qT.rearrange("(t p) n -> p t n", p=128)
        cT_r = cT.rearrange("(t p) n -> p t n", p=128)

        wv_sb = consts.tile([128, KT, 256], F32R, tag="wv")
        wq_sb = consts.tile([128, KT, 256], F32R, tag="wq")
        wo_sb = consts.tile([128, 2, C], F32R, tag="wo")

        def emit_ctx_block(pj, j, first=False):
            sl = slice(j * 512, (j + 1) * 512)
            cin = ins_pool.tile([128, KT, 512], F32R, tag="stage", name=f"cin{j}")
            if first:
                nc.sync.dma_start(out=cin[:, 0:2, :], in_=cT_r[:, 0:2, sl])
                nc.sync.dma_start(out=cin[:, 2:KT, :], in_=cT_r[:, 2:KT, sl])
            else:
                nc.sync.dma_start(out=cin, in_=cT_r[:, :, sl])
            for p in range(2):
                ps = pj.tile([128, 512], F32, tag="proj")
                for t in range(KT):
                    nc.tensor.matmul(
                        ps,
                        lhsT=wk_sb[:, t, p * 128:(p + 1) * 128],
                        rhs=cin[:, t, :],
                        start=(t == 0),
                        stop=(t == KT - 1),
                    )
                nc.vector.tensor_scalar_add(kt[p][:, sl], ps, bk_sb[:, p:p + 1])
            if first:
                nc.sync.dma_start(
                    out=wv_sb, in_=wv.rearrange("(t p) m -> p t m", p=128)
                )
            for s in range(4):
                i = j * 4 + s
                pv = pj.tile([128, 512], F32, tag="proj")
                for t in range(KT):
                    nc.tensor.matmul(
                        pv[:, 0:256],
                        lhsT=cin[:, t, s * 128:(s + 1) * 128],
                        rhs=wv_sb[:, t, :],
                        start=(t == 0),
                        stop=(t == KT - 1),
                    )
                nc.vector.memset(vt[i][:, :, D:D + 1].bitcast(F32), 1.0)
                nc.vector.tensor_add(
                    vt[i][:, :, 0:D],
                    pv[:, 0:256].rearrange("p (h d) -> p h d", h=HL),
                    bv_bc.rearrange("p (h d) -> p h d", h=HL),
                )

        def emit_q_block(pj, j):
            sl = slice(j * 512, (j + 1) * 512)
            hsl = slice((j % 2) * 512, (j % 2) * 512 + 512)
            qin = ins_pool.tile([128, KT, 512], F32R, tag="stage", name=f"qin{j}")
            nc.sync.dma_start(out=qin, in_=qT_r[:, :, sl])
            for p in range(2):
                ps = pj.tile([128, 512], F32, tag="proj")
                for t in range(KT):
                    nc.tensor.matmul(
                        ps,
                        lhsT=wq_sb[:, t, p * 128:(p + 1) * 128],
                        rhs=qin[:, t, :],
                        start=(t == 0),
                        stop=(t == KT - 1),
                    )
                nc.vector.tensor_scalar_add(
                    qtb[j // 2][p][:, hsl], ps, bq_sb[:, p:p + 1]
                )

        def emit_attention_head(ps_s, ps_ov, J, h, ot, weave=None):
            p, r = h // 2, h % 2
            rsl = slice(r * 64, (r + 1) * 64)
            ov = ps_ov.tile([65, 1024], F32, tag="ov")
            for i in range(T // 128):
                if weave and i in weave:
                    weave[i]()
                s = ps_s.tile([128, 1024], F32, tag="s")
                for half in range(2):
                    nc.tensor.matmul(
                        s[:, half * 512:(half + 1) * 512],
                        lhsT=kt[p][rsl, i * 128:(i + 1) * 128],
                        rhs=qtb[J][p][rsl, half * 512:(half + 1) * 512],
                        start=True,
                        stop=True,
                    )
                e = att.tile([128, 1024], F32R, tag="e")
                nc.scalar.activation(e, s, AF.Exp, scale=SCALE)
                for half in range(2):
                    nc.tensor.matmul(
                        ov[:, half * 512:(half + 1) * 512],
                        lhsT=vt[i][:, h, :],
                        rhs=e[:, half * 512:(half + 1) * 512],
                        start=(i == 0),
                        stop=(i == T // 128 - 1),
                    )
            # evict PSUM accumulator to SBUF immediately (frees the bank for
            # the next head), then normalize off the critical path:
            # rows 0..63 = unnormalized O^T, row 64 = softmax denominator.
            osb = smo.tile([65, 1024], F32, tag="osb")
            nc.vector.tensor_copy(osb, ov)
            rec = sm1.tile([1, 1024], F32, tag="rec")
            with nc.allow_low_precision(reason="f32 reciprocal"):
                nc.vector.reciprocal(rec, osb[64:65, :])
            bcast = sm1.tile([64, 1024], F32, tag="bcast")
            nc.gpsimd.partition_broadcast(bcast, rec)
            nc.vector.tensor_mul(ot[p][rsl, :], osb[0:64, :], bcast)

        def emit_outproj(ps_o, J, ot):
            for qi in range(8):
                qsl = slice(qi * 128, (qi + 1) * 128)
                ob = outs_pool.tile([128, 1024], F32, tag="ob")
                for ncol in range(2):
                    csl = slice(ncol * 512, (ncol + 1) * 512)
                    po = ps_o.tile([128, 512], F32, tag="po")
                    nc.tensor.matmul(
                        po, lhsT=ot[0][:, qsl], rhs=wo_sb[:, 0, csl],
                        start=True, stop=False,
                    )
                    nc.tensor.matmul(
                        po, lhsT=ot[1][:, qsl], rhs=wo_sb[:, 1, csl],
                        start=False, stop=True,
                    )
                    nc.vector.tensor_add(ob[:, csl], po, bo_bc[:, csl])
                nc.gpsimd.dma_start(
                    out=out[J * 1024 + qi * 128:J * 1024 + (qi + 1) * 128, :],
                    in_=ob,
                )

        # Emission order = dataflow order; later projections are woven into
        # attention head 0/1's ScalarE-paced chunk stream so the in-order PE
        # executes them inside attention's slack instead of before it.
        ps_s = ctx.enter_context(tc.tile_pool(name="ps_s", bufs=2, space="PSUM"))
        ps_ov = ctx.enter_context(tc.tile_pool(name="ps_ov", bufs=1, space="PSUM"))
        otJ = [[otp.tile([128, 1024], F32R, tag=f"ot{J}{p}", name=f"ot{J}{p}")
                for p in range(2)] for J in range(NB // 2)]
        pj_cm = tc.tile_pool(name="pj", bufs=2, space="PSUM")
        pj = pj_cm.__enter__()
        emit_ctx_block(pj, 0, first=True)
        nc.sync.dma_start(out=wq_sb, in_=wq.rearrange("(t p) m -> p t m", p=128))
        bq_sb = consts.tile([128, 2], F32, tag="bq")
        nc.gpsimd.dma_start(out=bq_sb, in_=bq.rearrange("(x p) -> p x", p=128))
        emit_q_block(pj, 0)
        emit_q_block(pj, 1)
        emit_attention_head(ps_s, ps_ov, 0, 0, otJ[0], weave={
            4: lambda: emit_ctx_block(pj, 1),
            8: lambda: emit_ctx_block(pj, 2),
            12: lambda: emit_ctx_block(pj, 3),
        })
        emit_attention_head(ps_s, ps_ov, 0, 1, otJ[0], weave={
            4: lambda: emit_q_block(pj, 2),
            10: lambda: emit_q_block(pj, 3),
        })
        pj_cm.__exit__(None, None, None)
        nc.sync.dma_start(out=wo_sb, in_=wo.rearrange("(t p) m -> p t m", p=128))
        bo_bc = consts.tile([128, C], F32, tag="bo")
        nc.gpsimd.dma_start(out=bo_bc, in_=_pbcast(bo4))
        with tc.tile_pool(name="ps_o", bufs=2, space="PSUM") as ps_o:
            emit_attention_head(ps_s, ps_ov, 0, 2, otJ[0])
            emit_attention_head(ps_s, ps_ov, 0, 3, otJ[0])
            emit_attention_head(ps_s, ps_ov, 1, 0, otJ[1])
            emit_outproj(ps_o, 0, otJ[0])
            emit_attention_head(ps_s, ps_ov, 1, 1, otJ[1])
            emit_attention_head(ps_s, ps_ov, 1, 2, otJ[1])
            emit_attention_head(ps_s, ps_ov, 1, 3, otJ[1])
            emit_outproj(ps_o, 1, otJ[1])


def build_program():
    global _PROGRAM
    if _PROGRAM is None:
        nc = bacc.Bacc(
            "TRN2", target_bir_lowering=False, debug=False, num_devices=8
        )
        with tile.TileContext(nc) as tc:
            _emit(tc)
        # Bacc.compile() legalizes to the TRN2 1-wait-per-instruction
        # constraint (generate_event_semaphores) among other passes.
        nc.compile()
        _PROGRAM = nc
    return _PROGRAM


def make_in_maps(query, context, Wq, bq, Wkv, bkv, Wo, bo):
    query = np.asarray(query, dtype=np.float32)
    context = np.asarray(context, dtype=np.float32)
    Wq = np.asarray(Wq, dtype=np.float32)
    bq = np.asarray(bq, dtype=np.float32)
    Wkv = np.asarray(Wkv, dtype=np.float32)
    bkv = np.asarray(bkv, dtype=np.float32)
    Wo = np.asarray(Wo, dtype=np.float32)
    bo = np.asarray(bo, dtype=np.float32)

    qTs = [np.ascontiguousarray(query[b].T) for b in range(2)]
    cTs = [np.ascontiguousarray(context[b].T) for b in range(2)]
    in_maps = []
    for c in range(8):
        b, hg = c // 4, c % 4
        cs = slice(hg * 256, (hg + 1) * 256)
        in_maps.append(
            {
                "qT": qTs[b],
                "cT": cTs[b],
                "wq": np.ascontiguousarray(Wq[:, cs]),
                "wk": np.ascontiguousarray(Wkv[:, cs]),
                "wv": np.ascontiguousarray(Wkv[:, 1024 + hg * 256:1024 + (hg + 1) * 256]),
                "wo": np.ascontiguousarray(Wo[cs, :]),
                "bq": np.ascontiguousarray(bq[cs]),
                "bk": np.ascontiguousarray(bkv[cs]),
                "bv": np.ascontiguousarray(bkv[1024 + hg * 256:1024 + (hg + 1) * 256]),
                "bo4": np.ascontiguousarray(bo * 0.25),
                "ones_in": np.ones(64, dtype=np.float32),
            }
        )
    return in_maps


def combine(parts):
    """parts: list of 8 [T, C] partials -> [2, T, C] full output."""
    out = np.empty((2, T, C), dtype=np.float32)
    for b in range(2):
        acc = parts[4 * b].astype(np.float32, copy=True)
        for c in range(4 * b + 1, 4 * b + 4):
            acc += parts[c]
        out[b] = acc
    return out


def kernel(**inputs):
    nc = build_program()
    in_maps = make_in_maps(**inputs)
    res = run_bass_kernel_spmd(nc, in_maps, list(range(8)))
    parts = [res.results[c]["out"] for c in range(8)]
    return combine(parts)

